# revision 4
# baseline (speedup 1.0000x reference)
"""Trainium2 Bass kernel for CoExDispProcessor (topk_masking).

Per-sample computation (data-parallel over batch across 8 cores):
  1. top-2 over the D=48 disparity axis of cost [1,48,128,240] -> softmax
     blend of the two indices -> disp4 [128,240]
  2. 3x3 unfold of disp4 (zero pad) -> nearest 4x upsample -> weighted sum
     with softmax over the 9 channels of spg [9,512,960] -> disp1 [512,960]

v3 layout/schedule:
  - cost loaded in 2 half DMAs on the sync queue ([128,48,128]/[128,48,112],
    512B/448B lines); spg streamed on the scalar queue (both HWDGE queues in
    parallel), interleaved with the exps so chunk-0 channels land early.
  - top-2 via MAX8 + MVL/FI8 per w column on DVE.
  - fine phase in 3 w-chunks of 320 fine cols: exp on ACT (fp16 out), den
    trees on Pool (plain adds only - broadcast muls on Pool contend badly
    with DVE SBUF access), num muls+adds+norm on DVE after the topk.
  - reciprocal on ACT with x0.25 scale folded in (gives 4/den); final
    num*r4 runs as a 2x fp16 TT with bf16 output (halves the output DMA).
  - disp4 chain split so the ACT texp dependency does not stall DVE.
"""

import os
import sys
from contextlib import ExitStack

import numpy as np

if "/opt/trn_rl_repo" not in sys.path:
    sys.path.insert(0, "/opt/trn_rl_repo")

import concourse.bass as bass
import concourse.bacc as bacc
import concourse.tile as tile
from concourse import mybir
from concourse.bass_utils import run_bass_kernel_spmd

F32 = mybir.dt.float32
BF16 = mybir.dt.bfloat16
FP16 = mybir.dt.float16
U16 = mybir.dt.uint16
OP = mybir.AluOpType
ACT = mybir.ActivationFunctionType

B, D, H, W = 8, 48, 128, 240
HF, WF = 4 * H, 4 * W  # 512, 960
N_CORES = 8
HALF_W = 128  # cost DMA halves / disp4 halves (512B-aligned slices)

NCH = 3  # fine chunks
FCH = WF // NCH  # fine columns per chunk (320)
WCH = FCH // 4  # coarse columns per chunk (80)

DEN_ENG = ["gpsimd", "gpsimd", "gpsimd"]
NEWTON = False  # refine the ACT reciprocal


def _act_reciprocal(nc, out_ap, in_ap, scale=1.0):
    eng = nc.scalar
    return eng.add_instruction(
        mybir.InstActivation(
            name=nc.get_next_instruction_name(),
            func=ACT.Reciprocal,
            ins=[
                eng.lower_ap(in_ap),
                mybir.ImmediateValue(dtype=F32, value=0.0),
                mybir.ImmediateValue(dtype=F32, value=float(scale)),
                mybir.ImmediateValue(dtype=F32, value=0.0),
            ],
            outs=[eng.lower_ap(out_ap)],
        )
    )


def build_kernel(ctx: ExitStack, tc: tile.TileContext, out_d, cost_d, spg_d):
    nc = tc.nc

    cost_hdw = cost_d.transpose([1, 0, 2])  # [128(h), 48(d), 240(w)] view
    spg_v = spg_d.rearrange("c (R dr) (k w) -> c R dr k w", dr=4, k=NCH)
    out_v = out_d.rearrange("(R dr) (k w) -> R dr k w", dr=4, k=NCH)

    persist = ctx.enter_context(tc.tile_pool(name="persist", bufs=1))
    small = ctx.enter_context(tc.tile_pool(name="small", bufs=1))
    raw_pool = ctx.enter_context(tc.tile_pool(name="raw", bufs=5))
    e_pool = ctx.enter_context(tc.tile_pool(name="epool", bufs=3))
    acc_pool = ctx.enter_context(tc.tile_pool(name="accp", bufs=1))
    out_pool = ctx.enter_context(tc.tile_pool(name="outp", bufs=2))

    # ---- persistent tiles -------------------------------------------------
    ctile = persist.tile([128, D, W], F32)
    v8 = persist.tile([128, W, 8], F32)
    i8 = persist.tile([128, W, 8], U16)
    i1f = small.tile([128, W], F32)
    i2f = small.tile([128, W], F32)
    delta = small.tile([128, W], F32)
    texp = small.tile([128, W], F32)
    numc = small.tile([128, W], F32)
    denc = small.tile([128, W], F32)
    rden = small.tile([128, W], F32)
    disp4 = small.tile([128, W], F32)
    rv = []
    urep = []
    for s in range(3):
        t = small.tile([128, W + 2], F32, tag=f"rv{s}")
        rv.append(t)
        nc.vector.memset(t[:], 0.0)
        u = small.tile([128, 4 * (W + 2)], FP16, tag=f"urep{s}")
        urep.append(u)

    # per-chunk fine tiles
    e_tiles = [
        e_pool.tile([128, 9, 4, FCH], FP16, tag="e", name=f"e{_k}")
        for _k in range(NCH)
    ]
    p_dve = acc_pool.tile([128, 2, 4 * FCH], FP16, tag="p_dve")
    dens = [
        acc_pool.tile([128, 4 * FCH], FP16, tag=f"den{k}", name=f"den{k}")
        for k in range(NCH)
    ]
    halves = [
        acc_pool.tile([128, 4 * FCH], FP16, tag=f"dh{k}", name=f"dh{k}")
        for k in range(NCH)
    ]
    nums = [
        acc_pool.tile([128, 4 * FCH], FP16, tag=f"num{k}", name=f"num{k}")
        for k in range(NCH)
    ]
    r4s = [
        acc_pool.tile([128, 4 * FCH], FP16, tag=f"r4_{k}", name=f"r4_{k}")
        for k in range(NCH)
    ]

    # ---- cost DMAs on the sync queue --------------------------------------
    nc.sync.dma_start(ctile[:, :, 0:HALF_W], cost_hdw[:, :, 0:HALF_W])
    nc.sync.dma_start(ctile[:, :, HALF_W:W], cost_hdw[:, :, HALF_W:W])

    # ---- spg DMAs on the scalar queue, interleaved with exps --------------
    raw_tiles = {}

    def spg_trigger(k, c):
        raw = raw_pool.tile([128, 4, FCH], F32, tag="raw", name=f"raw{k}_{c}")
        nc.scalar.dma_start(raw[:], spg_v[c, :, :, k, :])
        raw_tiles[(k, c)] = raw

    def exp_one(k, c):
        nc.scalar.activation(e_tiles[k][:, c], raw_tiles[(k, c)][:], ACT.Exp)

    def texp_half(h):
        a, b = (0, HALF_W) if h == 0 else (HALF_W, W)
        sl = slice(a, b)
        nc.scalar.activation(texp[:, sl], delta[:, sl], ACT.Exp)

    def urep_half(h):
        ua, ub = (0, 4 * (HALF_W + 1)) if h == 0 else (4 * (HALF_W + 1), 4 * (W + 2))
        ra, rb = (0, HALF_W + 1) if h == 0 else (HALF_W + 1, W + 2)
        for s in range(3):
            nc.scalar.copy(
                urep[s][:, ua:ub].rearrange("p (x dw) -> p x dw", dw=4),
                rv[s][:, ra:rb].unsqueeze(2).broadcast_to([128, rb - ra, 4]),
            )

    # ---- DVE: top-2 -------------------------------------------------------
    def maxes(a, b):
        for w in range(a, b):
            nc.vector.max(out=v8[:, w], in_=ctile[:, :, w])
        for w in range(a, b):
            nc.vector.max_index(i8[:, w], v8[:, w], ctile[:, :, w])

    def disp4_a(h):
        # produce delta for this half (feeds ACT texp); cheap DVE ops
        a, b = (0, HALF_W) if h == 0 else (HALF_W, W)
        sl = slice(a, b)
        nc.vector.tensor_copy(i1f[:, sl], i8[:, sl, 0])
        nc.vector.tensor_copy(i2f[:, sl], i8[:, sl, 1])
        nc.vector.tensor_sub(delta[:, sl], v8[:, sl, 1], v8[:, sl, 0])

    def disp4_b(h):
        a, b = (0, HALF_W) if h == 0 else (HALF_W, W)
        sl = slice(a, b)
        nc.vector.tensor_scalar_add(denc[:, sl], texp[:, sl], 1.0)
        nc.vector.tensor_mul(numc[:, sl], texp[:, sl], i2f[:, sl])
        nc.vector.reciprocal(rden[:, sl], denc[:, sl])
        nc.vector.tensor_add(numc[:, sl], numc[:, sl], i1f[:, sl])
        nc.vector.tensor_mul(disp4[:, sl], numc[:, sl], rden[:, sl])
        # rv[s][r, 1+w] = disp4[r + s - 1, w] for this half's columns
        nc.vector.tensor_copy(rv[1][:, 1 + a:1 + b], disp4[:, sl])
        nc.sync.dma_start(rv[0][1:128, 1 + a:1 + b], disp4[0:127, sl])
        nc.sync.dma_start(rv[2][0:127, 1 + a:1 + b], disp4[1:128, sl])

    # ---- fine phase per chunk ---------------------------------------------
    def u4(k, c):
        cirow, cj = c // 3, c % 3
        off = 4 * (cj + k * WCH)
        return (urep[cirow][:, off:off + FCH]
                .unsqueeze(1).broadcast_to([128, 4, FCH]))

    def den_chunk(k, eng_name):
        """den = sum_c e_c; two independent serial chains then merge."""
        eng = getattr(nc, eng_name)
        e = e_tiles[k]
        den = dens[k]
        half = halves[k]
        ef = lambda c: e[:, c].rearrange("p a b -> p (a b)")
        eng.tensor_add(den[:], ef(0), ef(1))
        eng.tensor_add(half[:], ef(2), ef(3))
        eng.tensor_add(den[:], den[:], ef(4))
        eng.tensor_add(half[:], half[:], ef(5))
        eng.tensor_add(den[:], den[:], ef(6))
        eng.tensor_add(half[:], half[:], ef(7))
        eng.tensor_add(den[:], den[:], ef(8))
        eng.tensor_add(den[:], den[:], half[:])
        # reciprocal with x4 folded: r4 = 1/(den*0.25) = 4/den
        _act_reciprocal(nc, r4s[k][:], den[:], scale=0.25)
        if NEWTON:
            m = halves[k]
            nc.vector.tensor_mul(m[:], dens[k][:], r4s[k][:])
            nc.vector.tensor_scalar(m[:], m[:], -0.25, 2.0, op0=OP.mult, op1=OP.add)
            nc.vector.tensor_mul(r4s[k][:], r4s[k][:], m[:])

    def num_chunk(k):
        """num = sum_c e_c * u4_c on DVE with ping-pong product buffers."""
        p = p_dve
        e = e_tiles[k]
        num = nums[k]
        p3 = lambda i: p[:, i].rearrange("p (a b) -> p a b", a=4)
        nc.vector.tensor_mul(p3(0), e[:, 0], u4(k, 0))
        nc.vector.tensor_mul(p3(1), e[:, 1], u4(k, 1))
        nc.vector.tensor_add(num[:], p[:, 0], p[:, 1])
        for c in range(2, 9):
            nc.vector.tensor_mul(p3(c % 2), e[:, c], u4(k, c))
            nc.vector.tensor_add(num[:], num[:], p[:, c % 2])

    def norm_chunk(k):
        outt = out_pool.tile([128, 4, FCH], BF16, tag="outt", name=f"outt{k}")
        nc.vector.tensor_mul(
            outt[:].rearrange("p a b -> p (a b)"), nums[k][:], r4s[k][:]
        )
        nc.sync.dma_start(out_v[:, :, k, :], outt[:])

    # ---- schedule ---------------------------------------------------------
    # scalar queue: trigger chunk-0 spg first, then exp as each channel lands,
    # keeping a 2-transfer trigger lead.
    for c in range(2):
        spg_trigger(0, c)
    for c in range(9):
        if c + 2 < 9:
            spg_trigger(0, c + 2)
        else:
            spg_trigger(1, c + 2 - 9)
        exp_one(0, c)
    for c in range(9):
        if c + 2 < 9:
            spg_trigger(1, c + 2)
        else:
            spg_trigger(2, c + 2 - 9)
        exp_one(1, c)

    maxes(0, HALF_W)
    disp4_a(0)
    den_chunk(0, DEN_ENG[0])  # Pool, overlaps DVE right-half topk
    texp_half(0)
    maxes(HALF_W, W)
    disp4_b(0)
    urep_half(0)
    disp4_a(1)
    den_chunk(1, DEN_ENG[1])
    num_chunk(0)
    texp_half(1)
    disp4_b(1)
    urep_half(1)
    norm_chunk(0)
    for c in range(2, 9):
        spg_trigger(2, c)
    for c in range(9):
        exp_one(2, c)
    num_chunk(1)
    den_chunk(2, DEN_ENG[2])
    norm_chunk(1)
    num_chunk(2)
    norm_chunk(2)


def build_program():
    nc = bacc.Bacc(
        "TRN2",
        target_bir_lowering=False,
        debug=False,
        enable_asserts=False,
        num_devices=N_CORES,
    )
    cost_d = nc.dram_tensor("cost", [D, H, W], F32, kind="ExternalInput").ap()
    spg_d = nc.dram_tensor("spg", [9, HF, WF], F32, kind="ExternalInput").ap()
    out_d = nc.dram_tensor("out", [HF, WF], BF16, kind="ExternalOutput").ap()
    with tile.TileContext(nc) as tc:
        with ExitStack() as ctx:
            build_kernel(ctx, tc, out_d, cost_d, spg_d)
    nc.compile()
    return nc


def _install_ntff_hook():
    """Provide antenv.axon_hooks + register the ctypes NTFF profiler."""
    import types

    if "antenv.axon_hooks" in sys.modules:
        return True
    try:
        import antenv
        from trn_agent_boot.trn_boot import _ntff_profile_via_ctypes

        mod = types.ModuleType("antenv.axon_hooks")
        mod._hook = None

        def set_axon_ntff_profile_hook(hook):
            mod._hook = hook

        def get_axon_ntff_profile_hook():
            return mod._hook

        mod.set_axon_ntff_profile_hook = set_axon_ntff_profile_hook
        mod.get_axon_ntff_profile_hook = get_axon_ntff_profile_hook
        sys.modules["antenv.axon_hooks"] = mod
        antenv.axon_hooks = mod
        mod._hook = _ntff_profile_via_ctypes("/opt/axon/libaxon_pjrt.so")
        return True
    except Exception as e:  # profiling is best-effort
        print(f"NTFF hook install failed: {e}")
        return False


LAST_RESULTS = None


def kernel(cost: np.ndarray, spg: np.ndarray) -> np.ndarray:
    """cost [8,1,48,128,240] f32, spg [8,9,512,960] f32 -> disp1 [8,512,960] f32."""
    global LAST_RESULTS
    cost = np.ascontiguousarray(np.asarray(cost, dtype=np.float32))
    spg = np.ascontiguousarray(np.asarray(spg, dtype=np.float32))
    assert cost.shape == (B, 1, D, H, W) and spg.shape == (B, 9, HF, WF)

    nc = build_program()
    in_maps = [
        {"cost": cost[b, 0], "spg": spg[b]} for b in range(B)
    ]
    trace = bool(int(os.environ.get("KERNEL_TRACE", "0")))
    if trace:
        trace = _install_ntff_hook()
    res = run_bass_kernel_spmd(
        nc, in_maps, core_ids=list(range(N_CORES)), trace=trace
    )
    LAST_RESULTS = res
    out = np.stack(
        [np.asarray(res.results[b]["out"]) for b in range(B)], axis=0
    )
    return out.astype(np.float32, copy=False)


# revision 5
# speedup vs baseline: 1.0383x; 1.0383x over previous
"""Trainium2 Bass kernel for CoExDispProcessor (topk_masking).

Per-sample computation (data-parallel over batch across 8 cores):
  1. top-2 over the D=48 disparity axis of cost [1,48,128,240] -> softmax
     blend of the two indices -> disp4 [128,240]
  2. 3x3 unfold of disp4 (zero pad) -> nearest 4x upsample -> weighted sum
     with softmax over the 9 channels of spg [9,512,960] -> disp1 [512,960]

v3 layout/schedule:
  - cost loaded in 2 half DMAs on the sync queue ([128,48,128]/[128,48,112],
    512B/448B lines); spg streamed on the scalar queue (both HWDGE queues in
    parallel), interleaved with the exps so chunk-0 channels land early.
  - top-2 via MAX8 + MVL/FI8 per w column on DVE.
  - fine phase in 3 w-chunks of 320 fine cols: exp on ACT (fp16 out), den
    trees on Pool (plain adds only - broadcast muls on Pool contend badly
    with DVE SBUF access), num muls+adds+norm on DVE after the topk.
  - reciprocal on ACT with x0.25 scale folded in (gives 4/den); final
    num*r4 runs as a 2x fp16 TT with bf16 output (halves the output DMA).
  - disp4 chain split so the ACT texp dependency does not stall DVE.
"""

import os
import sys
from contextlib import ExitStack

import numpy as np

if "/opt/trn_rl_repo" not in sys.path:
    sys.path.insert(0, "/opt/trn_rl_repo")

import concourse.bass as bass
import concourse.bacc as bacc
import concourse.tile as tile
from concourse import mybir
from concourse.bass_utils import run_bass_kernel_spmd

F32 = mybir.dt.float32
BF16 = mybir.dt.bfloat16
FP16 = mybir.dt.float16
U16 = mybir.dt.uint16
OP = mybir.AluOpType
ACT = mybir.ActivationFunctionType

B, D, H, W = 8, 48, 128, 240
HF, WF = 4 * H, 4 * W  # 512, 960
N_CORES = 8
HALF_W = 128  # cost DMA halves / disp4 halves (512B-aligned slices)

NCH = 3  # fine chunks
FCH = WF // NCH  # fine columns per chunk (320)
WCH = FCH // 4  # coarse columns per chunk (80)

DEN_ENG = ["gpsimd", "gpsimd", "gpsimd"]
NEWTON = False  # refine the ACT reciprocal


def _act_reciprocal(nc, out_ap, in_ap, scale=1.0):
    eng = nc.scalar
    return eng.add_instruction(
        mybir.InstActivation(
            name=nc.get_next_instruction_name(),
            func=ACT.Reciprocal,
            ins=[
                eng.lower_ap(in_ap),
                mybir.ImmediateValue(dtype=F32, value=0.0),
                mybir.ImmediateValue(dtype=F32, value=float(scale)),
                mybir.ImmediateValue(dtype=F32, value=0.0),
            ],
            outs=[eng.lower_ap(out_ap)],
        )
    )


def build_kernel(ctx: ExitStack, tc: tile.TileContext, out_d, cost_d, spg_d):
    nc = tc.nc

    cost_hdw = cost_d.transpose([1, 0, 2])  # [128(h), 48(d), 240(w)] view
    spg_v = spg_d.rearrange("c (R dr) (k w) -> c R dr k w", dr=4, k=NCH)
    out_v = out_d.rearrange("(R dr) (k w) -> R dr k w", dr=4, k=NCH)

    persist = ctx.enter_context(tc.tile_pool(name="persist", bufs=1))
    small = ctx.enter_context(tc.tile_pool(name="small", bufs=1))
    raw_pool = ctx.enter_context(tc.tile_pool(name="raw", bufs=5))
    e_pool = ctx.enter_context(tc.tile_pool(name="epool", bufs=3))
    acc_pool = ctx.enter_context(tc.tile_pool(name="accp", bufs=1))
    out_pool = ctx.enter_context(tc.tile_pool(name="outp", bufs=2))

    # ---- persistent tiles -------------------------------------------------
    ctile = persist.tile([128, D, W], F32)
    v8 = persist.tile([128, W, 8], F32)
    i8 = persist.tile([128, W, 8], U16)
    i1f = small.tile([128, W], F32)
    i2f = small.tile([128, W], F32)
    delta = small.tile([128, W], F32)
    texp = small.tile([128, W], F32)
    numc = small.tile([128, W], F32)
    denc = small.tile([128, W], F32)
    rden = small.tile([128, W], F32)
    disp4 = small.tile([128, W], F32)
    rv = []
    urep = []
    for s in range(3):
        t = small.tile([128, W + 2], F32, tag=f"rv{s}")
        rv.append(t)
        nc.vector.memset(t[:], 0.0)
        u = small.tile([128, 4 * (W + 2)], FP16, tag=f"urep{s}")
        urep.append(u)

    # per-chunk fine tiles
    e_tiles = [
        e_pool.tile([128, 9, 4, FCH], FP16, tag="e", name=f"e{_k}")
        for _k in range(NCH)
    ]
    p_dve = acc_pool.tile([128, 2, 4 * FCH], FP16, tag="p_dve")
    dens = [
        acc_pool.tile([128, 4 * FCH], FP16, tag=f"den{k}", name=f"den{k}")
        for k in range(NCH)
    ]
    halves = [
        acc_pool.tile([128, 4 * FCH], FP16, tag=f"dh{k}", name=f"dh{k}")
        for k in range(NCH)
    ]
    nums = [
        acc_pool.tile([128, 4 * FCH], FP16, tag=f"num{k}", name=f"num{k}")
        for k in range(NCH)
    ]
    r4s = [
        acc_pool.tile([128, 4 * FCH], FP16, tag=f"r4_{k}", name=f"r4_{k}")
        for k in range(NCH)
    ]

    # ---- cost DMAs: two quarters per queue so the halves stream
    # concurrently (the strided pattern caps a single stream at ~90-140GB/s)
    nc.sync.dma_start(ctile[:, :, 0:64], cost_hdw[:, :, 0:64])
    nc.gpsimd.dma_start(ctile[:, :, HALF_W:192], cost_hdw[:, :, HALF_W:192])
    nc.sync.dma_start(ctile[:, :, 64:HALF_W], cost_hdw[:, :, 64:HALF_W])
    nc.gpsimd.dma_start(ctile[:, :, 192:W], cost_hdw[:, :, 192:W])

    # ---- spg DMAs on the scalar queue, interleaved with exps --------------
    raw_tiles = {}

    def spg_trigger(k, c):
        raw = raw_pool.tile([128, 4, FCH], F32, tag="raw", name=f"raw{k}_{c}")
        nc.scalar.dma_start(raw[:], spg_v[c, :, :, k, :])
        raw_tiles[(k, c)] = raw

    def exp_one(k, c):
        nc.scalar.activation(e_tiles[k][:, c], raw_tiles[(k, c)][:], ACT.Exp)

    def texp_half(h):
        a, b = (0, HALF_W) if h == 0 else (HALF_W, W)
        sl = slice(a, b)
        nc.scalar.activation(texp[:, sl], delta[:, sl], ACT.Exp)

    def urep_half(h):
        ua, ub = (0, 4 * (HALF_W + 1)) if h == 0 else (4 * (HALF_W + 1), 4 * (W + 2))
        ra, rb = (0, HALF_W + 1) if h == 0 else (HALF_W + 1, W + 2)
        for s in range(3):
            nc.scalar.copy(
                urep[s][:, ua:ub].rearrange("p (x dw) -> p x dw", dw=4),
                rv[s][:, ra:rb].unsqueeze(2).broadcast_to([128, rb - ra, 4]),
            )

    # ---- DVE: top-2 -------------------------------------------------------
    def maxes(a, b):
        for w in range(a, b):
            nc.vector.max(out=v8[:, w], in_=ctile[:, :, w])
        for w in range(a, b):
            nc.vector.max_index(i8[:, w], v8[:, w], ctile[:, :, w])

    def disp4_a(h):
        # produce delta for this half (feeds ACT texp); cheap DVE ops
        a, b = (0, HALF_W) if h == 0 else (HALF_W, W)
        sl = slice(a, b)
        nc.vector.tensor_copy(i1f[:, sl], i8[:, sl, 0])
        nc.vector.tensor_copy(i2f[:, sl], i8[:, sl, 1])
        nc.vector.tensor_sub(delta[:, sl], v8[:, sl, 1], v8[:, sl, 0])

    def disp4_b(h):
        a, b = (0, HALF_W) if h == 0 else (HALF_W, W)
        sl = slice(a, b)
        nc.vector.tensor_scalar_add(denc[:, sl], texp[:, sl], 1.0)
        nc.vector.tensor_mul(numc[:, sl], texp[:, sl], i2f[:, sl])
        nc.vector.reciprocal(rden[:, sl], denc[:, sl])
        nc.vector.tensor_add(numc[:, sl], numc[:, sl], i1f[:, sl])
        nc.vector.tensor_mul(disp4[:, sl], numc[:, sl], rden[:, sl])
        # rv[s][r, 1+w] = disp4[r + s - 1, w] for this half's columns
        nc.vector.tensor_copy(rv[1][:, 1 + a:1 + b], disp4[:, sl])
        nc.sync.dma_start(rv[0][1:128, 1 + a:1 + b], disp4[0:127, sl])
        nc.sync.dma_start(rv[2][0:127, 1 + a:1 + b], disp4[1:128, sl])

    # ---- fine phase per chunk ---------------------------------------------
    def u4(k, c):
        cirow, cj = c // 3, c % 3
        off = 4 * (cj + k * WCH)
        return (urep[cirow][:, off:off + FCH]
                .unsqueeze(1).broadcast_to([128, 4, FCH]))

    def den_chunk(k, eng_name):
        """den = sum_c e_c; two independent serial chains then merge."""
        eng = getattr(nc, eng_name)
        e = e_tiles[k]
        den = dens[k]
        half = halves[k]
        ef = lambda c: e[:, c].rearrange("p a b -> p (a b)")
        eng.tensor_add(den[:], ef(0), ef(1))
        eng.tensor_add(half[:], ef(2), ef(3))
        eng.tensor_add(den[:], den[:], ef(4))
        eng.tensor_add(half[:], half[:], ef(5))
        eng.tensor_add(den[:], den[:], ef(6))
        eng.tensor_add(half[:], half[:], ef(7))
        eng.tensor_add(den[:], den[:], ef(8))
        eng.tensor_add(den[:], den[:], half[:])

    def recip_chunk(k):
        # reciprocal with x4 folded: r4 = 1/(den*0.25) = 4/den
        _act_reciprocal(nc, r4s[k][:], dens[k][:], scale=0.25)
        if NEWTON:
            m = halves[k]
            nc.vector.tensor_mul(m[:], dens[k][:], r4s[k][:])
            nc.vector.tensor_scalar(m[:], m[:], -0.25, 2.0, op0=OP.mult, op1=OP.add)
            nc.vector.tensor_mul(r4s[k][:], r4s[k][:], m[:])

    def num_chunk(k):
        """num = sum_c e_c * u4_c on DVE with ping-pong product buffers."""
        p = p_dve
        e = e_tiles[k]
        num = nums[k]
        p3 = lambda i: p[:, i].rearrange("p (a b) -> p a b", a=4)
        nc.vector.tensor_mul(p3(0), e[:, 0], u4(k, 0))
        nc.vector.tensor_mul(p3(1), e[:, 1], u4(k, 1))
        nc.vector.tensor_add(num[:], p[:, 0], p[:, 1])
        for c in range(2, 9):
            nc.vector.tensor_mul(p3(c % 2), e[:, c], u4(k, c))
            nc.vector.tensor_add(num[:], num[:], p[:, c % 2])

    def norm_chunk(k):
        outt = out_pool.tile([128, 4, FCH], BF16, tag="outt", name=f"outt{k}")
        nc.vector.tensor_mul(
            outt[:].rearrange("p a b -> p (a b)"), nums[k][:], r4s[k][:]
        )
        nc.sync.dma_start(out_v[:, :, k, :], outt[:])

    # ---- schedule ---------------------------------------------------------
    # scalar queue: trigger chunk-0 spg first, then exp as each channel lands,
    # keeping a 2-transfer trigger lead.
    for c in range(2):
        spg_trigger(0, c)
    for c in range(9):
        if c + 2 < 9:
            spg_trigger(0, c + 2)
        else:
            spg_trigger(1, c + 2 - 9)
        exp_one(0, c)
    for c in range(9):
        if c + 2 < 9:
            spg_trigger(1, c + 2)
        else:
            spg_trigger(2, c + 2 - 9)
        exp_one(1, c)

    maxes(0, 64)
    den_chunk(0, DEN_ENG[0])  # Pool, overlaps DVE topk
    maxes(64, HALF_W)
    disp4_a(0)
    texp_half(0)
    maxes(HALF_W, 192)
    disp4_b(0)
    urep_half(0)
    den_chunk(1, DEN_ENG[1])
    maxes(192, W)
    disp4_a(1)
    texp_half(1)
    disp4_b(1)
    urep_half(1)
    for c in range(2, 9):
        spg_trigger(2, c)
    for c in range(9):
        exp_one(2, c)
    den_chunk(2, DEN_ENG[2])
    recip_chunk(0)
    recip_chunk(1)
    num_chunk(0)
    norm_chunk(0)
    num_chunk(1)
    recip_chunk(2)
    norm_chunk(1)
    num_chunk(2)
    norm_chunk(2)


def build_program():
    nc = bacc.Bacc(
        "TRN2",
        target_bir_lowering=False,
        debug=False,
        enable_asserts=False,
        num_devices=N_CORES,
    )
    cost_d = nc.dram_tensor("cost", [D, H, W], F32, kind="ExternalInput").ap()
    spg_d = nc.dram_tensor("spg", [9, HF, WF], F32, kind="ExternalInput").ap()
    out_d = nc.dram_tensor("out", [HF, WF], BF16, kind="ExternalOutput").ap()
    with tile.TileContext(nc) as tc:
        with ExitStack() as ctx:
            build_kernel(ctx, tc, out_d, cost_d, spg_d)
    nc.compile()
    return nc


def _install_ntff_hook():
    """Provide antenv.axon_hooks + register the ctypes NTFF profiler."""
    import types

    if "antenv.axon_hooks" in sys.modules:
        return True
    try:
        import antenv
        from trn_agent_boot.trn_boot import _ntff_profile_via_ctypes

        mod = types.ModuleType("antenv.axon_hooks")
        mod._hook = None

        def set_axon_ntff_profile_hook(hook):
            mod._hook = hook

        def get_axon_ntff_profile_hook():
            return mod._hook

        mod.set_axon_ntff_profile_hook = set_axon_ntff_profile_hook
        mod.get_axon_ntff_profile_hook = get_axon_ntff_profile_hook
        sys.modules["antenv.axon_hooks"] = mod
        antenv.axon_hooks = mod
        mod._hook = _ntff_profile_via_ctypes("/opt/axon/libaxon_pjrt.so")
        return True
    except Exception as e:  # profiling is best-effort
        print(f"NTFF hook install failed: {e}")
        return False


LAST_RESULTS = None


def kernel(cost: np.ndarray, spg: np.ndarray) -> np.ndarray:
    """cost [8,1,48,128,240] f32, spg [8,9,512,960] f32 -> disp1 [8,512,960] f32."""
    global LAST_RESULTS
    cost = np.ascontiguousarray(np.asarray(cost, dtype=np.float32))
    spg = np.ascontiguousarray(np.asarray(spg, dtype=np.float32))
    assert cost.shape == (B, 1, D, H, W) and spg.shape == (B, 9, HF, WF)

    nc = build_program()
    in_maps = [
        {"cost": cost[b, 0], "spg": spg[b]} for b in range(B)
    ]
    trace = bool(int(os.environ.get("KERNEL_TRACE", "0")))
    if trace:
        trace = _install_ntff_hook()
    res = run_bass_kernel_spmd(
        nc, in_maps, core_ids=list(range(N_CORES)), trace=trace
    )
    LAST_RESULTS = res
    out = np.stack(
        [np.asarray(res.results[b]["out"]) for b in range(B)], axis=0
    )
    return out.astype(np.float32, copy=False)


# revision 8
# speedup vs baseline: 1.1051x; 1.0644x over previous
"""Trainium2 Bass kernel for CoExDispProcessor (topk_masking).

Per-sample computation (data-parallel over batch across 8 cores):
  1. top-2 over the D=48 disparity axis of cost [1,48,128,240] -> softmax
     blend of the two indices -> disp4 [128,240]
  2. 3x3 unfold of disp4 (zero pad) -> nearest 4x upsample -> weighted sum
     with softmax over the 9 channels of spg [9,512,960] -> disp1 [512,960]

v3 layout/schedule:
  - cost loaded in 2 half DMAs on the sync queue ([128,48,128]/[128,48,112],
    512B/448B lines); spg streamed on the scalar queue (both HWDGE queues in
    parallel), interleaved with the exps so chunk-0 channels land early.
  - top-2 via MAX8 + MVL/FI8 per w column on DVE.
  - fine phase in 3 w-chunks of 320 fine cols: exp on ACT (fp16 out), den
    trees on Pool (plain adds only - broadcast muls on Pool contend badly
    with DVE SBUF access), num muls+adds+norm on DVE after the topk.
  - reciprocal on ACT with x0.25 scale folded in (gives 4/den); final
    num*r4 runs as a 2x fp16 TT with bf16 output (halves the output DMA).
  - disp4 chain split so the ACT texp dependency does not stall DVE.
"""

import os
import sys
from contextlib import ExitStack

import numpy as np

if "/opt/trn_rl_repo" not in sys.path:
    sys.path.insert(0, "/opt/trn_rl_repo")

import concourse.bass as bass
import concourse.bacc as bacc
import concourse.tile as tile
from concourse import mybir
from concourse.bass_utils import run_bass_kernel_spmd

F32 = mybir.dt.float32
BF16 = mybir.dt.bfloat16
FP16 = mybir.dt.float16
U16 = mybir.dt.uint16
OP = mybir.AluOpType
ACT = mybir.ActivationFunctionType

B, D, H, W = 8, 48, 128, 240
HF, WF = 4 * H, 4 * W  # 512, 960
N_CORES = 8
HALF_W = 128  # cost DMA halves / disp4 halves (512B-aligned slices)

NCH = 3  # fine chunks
FCH = WF // NCH  # fine columns per chunk (320)
WCH = FCH // 4  # coarse columns per chunk (80)

DEN_ENG = ["gpsimd", "gpsimd", "gpsimd"]
NEWTON = False  # refine the ACT reciprocal


def _act_reciprocal(nc, out_ap, in_ap, scale=1.0):
    eng = nc.scalar
    return eng.add_instruction(
        mybir.InstActivation(
            name=nc.get_next_instruction_name(),
            func=ACT.Reciprocal,
            ins=[
                eng.lower_ap(in_ap),
                mybir.ImmediateValue(dtype=F32, value=0.0),
                mybir.ImmediateValue(dtype=F32, value=float(scale)),
                mybir.ImmediateValue(dtype=F32, value=0.0),
            ],
            outs=[eng.lower_ap(out_ap)],
        )
    )


def build_kernel(ctx: ExitStack, tc: tile.TileContext, out_d, cost_d, spg_d):
    nc = tc.nc

    cost_hdw = cost_d.transpose([1, 0, 2])  # [128(h), 48(d), 240(w)] view
    spg_v = spg_d.rearrange("c (R dr) (k w) -> c R dr k w", dr=4, k=NCH)
    out_v = out_d.rearrange("(R dr) (k w) -> R dr k w", dr=4, k=NCH)

    persist = ctx.enter_context(tc.tile_pool(name="persist", bufs=1))
    small = ctx.enter_context(tc.tile_pool(name="small", bufs=1))
    raw_pool = ctx.enter_context(tc.tile_pool(name="raw", bufs=5))
    e_pool = ctx.enter_context(tc.tile_pool(name="epool", bufs=3))
    acc_pool = ctx.enter_context(tc.tile_pool(name="accp", bufs=1))
    out_pool = ctx.enter_context(tc.tile_pool(name="outp", bufs=2))

    # ---- persistent tiles -------------------------------------------------
    ctile = persist.tile([128, D, W], F32)
    v8 = persist.tile([128, W, 8], F32)
    i8 = persist.tile([128, W, 8], U16)
    i1f = small.tile([128, W], F32)
    i2f = small.tile([128, W], F32)
    delta = small.tile([128, W], F32)
    texp = small.tile([128, W], F32)
    numc = small.tile([128, W], F32)
    denc = small.tile([128, W], F32)
    rden = small.tile([128, W], F32)
    disp4 = small.tile([128, W], F32)
    rv = []
    urep = []
    for s in range(3):
        t = small.tile([128, W + 2], F32, tag=f"rv{s}")
        rv.append(t)
        nc.vector.memset(t[:], 0.0)
        u = small.tile([128, 4 * (W + 2)], FP16, tag=f"urep{s}")
        urep.append(u)

    # per-chunk fine tiles
    e_tiles = [
        e_pool.tile([128, 9, 4, FCH], FP16, tag="e", name=f"e{_k}")
        for _k in range(NCH)
    ]
    p_dve = acc_pool.tile([128, 2, 4 * FCH], FP16, tag="p_dve")
    dens = [
        acc_pool.tile([128, 4 * FCH], FP16, tag=f"den{k}", name=f"den{k}")
        for k in range(NCH)
    ]
    halves = [
        acc_pool.tile([128, 4 * FCH], FP16, tag=f"dh{k}", name=f"dh{k}")
        for k in range(NCH)
    ]
    nums = [
        acc_pool.tile([128, 4 * FCH], FP16, tag=f"num{k}", name=f"num{k}")
        for k in range(NCH)
    ]
    r4s = [
        acc_pool.tile([128, 4 * FCH], FP16, tag=f"r4_{k}", name=f"r4_{k}")
        for k in range(NCH)
    ]

    # ---- cost DMAs: 4 quarters on the gpsimd SWDGE queue. HWDGE blocks the
    # issuing engine for the whole transfer on this many-descriptor strided
    # pattern; SWDGE generates descriptors in ~2.7us and streams async.
    nc.gpsimd.dma_start(ctile[:, :, 0:64], cost_hdw[:, :, 0:64])
    nc.gpsimd.dma_start(ctile[:, :, 64:HALF_W], cost_hdw[:, :, 64:HALF_W])
    nc.gpsimd.dma_start(ctile[:, :, HALF_W:192], cost_hdw[:, :, HALF_W:192])
    nc.gpsimd.dma_start(ctile[:, :, 192:W], cost_hdw[:, :, 192:W])

    # ---- spg DMAs on the scalar queue, interleaved with exps --------------
    raw_tiles = {}

    def spg_trigger(k, c):
        raw = raw_pool.tile([128, 4, FCH], F32, tag="raw", name=f"raw{k}_{c}")
        nc.scalar.dma_start(raw[:], spg_v[c, :, :, k, :])
        raw_tiles[(k, c)] = raw

    def exp_one(k, c):
        nc.scalar.activation(e_tiles[k][:, c], raw_tiles[(k, c)][:], ACT.Exp)

    def texp_half(h):
        a, b = (0, HALF_W) if h == 0 else (HALF_W, W)
        sl = slice(a, b)
        nc.scalar.activation(texp[:, sl], delta[:, sl], ACT.Exp)

    def urep_half(h):
        ua, ub = (0, 4 * (HALF_W + 1)) if h == 0 else (4 * (HALF_W + 1), 4 * (W + 2))
        ra, rb = (0, HALF_W + 1) if h == 0 else (HALF_W + 1, W + 2)
        for s in range(3):
            nc.scalar.copy(
                urep[s][:, ua:ub].rearrange("p (x dw) -> p x dw", dw=4),
                rv[s][:, ra:rb].unsqueeze(2).broadcast_to([128, rb - ra, 4]),
            )

    # ---- DVE: top-2 -------------------------------------------------------
    def maxes(a, b):
        for w in range(a, b):
            nc.vector.max(out=v8[:, w], in_=ctile[:, :, w])
        for w in range(a, b):
            nc.vector.max_index(i8[:, w], v8[:, w], ctile[:, :, w])

    def disp4_a(h):
        # produce delta for this half (feeds ACT texp); cheap DVE ops
        a, b = (0, HALF_W) if h == 0 else (HALF_W, W)
        sl = slice(a, b)
        nc.vector.tensor_copy(i1f[:, sl], i8[:, sl, 0])
        nc.vector.tensor_copy(i2f[:, sl], i8[:, sl, 1])
        nc.vector.tensor_sub(delta[:, sl], v8[:, sl, 1], v8[:, sl, 0])

    def disp4_b(h):
        a, b = (0, HALF_W) if h == 0 else (HALF_W, W)
        sl = slice(a, b)
        nc.vector.tensor_scalar_add(denc[:, sl], texp[:, sl], 1.0)
        nc.vector.tensor_mul(numc[:, sl], texp[:, sl], i2f[:, sl])
        nc.vector.reciprocal(rden[:, sl], denc[:, sl])
        nc.vector.tensor_add(numc[:, sl], numc[:, sl], i1f[:, sl])
        nc.vector.tensor_mul(disp4[:, sl], numc[:, sl], rden[:, sl])
        # rv[s][r, 1+w] = disp4[r + s - 1, w] for this half's columns
        nc.vector.tensor_copy(rv[1][:, 1 + a:1 + b], disp4[:, sl])
        nc.sync.dma_start(rv[0][1:128, 1 + a:1 + b], disp4[0:127, sl])
        nc.sync.dma_start(rv[2][0:127, 1 + a:1 + b], disp4[1:128, sl])

    # ---- fine phase per chunk ---------------------------------------------
    def u4(k, c):
        cirow, cj = c // 3, c % 3
        off = 4 * (cj + k * WCH)
        return (urep[cirow][:, off:off + FCH]
                .unsqueeze(1).broadcast_to([128, 4, FCH]))

    def den_chunk(k, eng_name):
        """den = sum_c e_c; two independent serial chains then merge."""
        eng = getattr(nc, eng_name)
        e = e_tiles[k]
        den = dens[k]
        half = halves[k]
        ef = lambda c: e[:, c].rearrange("p a b -> p (a b)")
        eng.tensor_add(den[:], ef(0), ef(1))
        eng.tensor_add(half[:], ef(2), ef(3))
        eng.tensor_add(den[:], den[:], ef(4))
        eng.tensor_add(half[:], half[:], ef(5))
        eng.tensor_add(den[:], den[:], ef(6))
        eng.tensor_add(half[:], half[:], ef(7))
        eng.tensor_add(den[:], den[:], ef(8))
        eng.tensor_add(den[:], den[:], half[:])

    def recip_chunk(k):
        # reciprocal with x4 folded: r4 = 1/(den*0.25) = 4/den
        _act_reciprocal(nc, r4s[k][:], dens[k][:], scale=0.25)
        if NEWTON:
            m = halves[k]
            nc.vector.tensor_mul(m[:], dens[k][:], r4s[k][:])
            nc.vector.tensor_scalar(m[:], m[:], -0.25, 2.0, op0=OP.mult, op1=OP.add)
            nc.vector.tensor_mul(r4s[k][:], r4s[k][:], m[:])

    def num_chunk(k):
        """num = sum_c e_c * u4_c on DVE with ping-pong product buffers."""
        p = p_dve
        e = e_tiles[k]
        num = nums[k]
        p3 = lambda i: p[:, i].rearrange("p (a b) -> p a b", a=4)
        nc.vector.tensor_mul(p3(0), e[:, 0], u4(k, 0))
        nc.vector.tensor_mul(p3(1), e[:, 1], u4(k, 1))
        nc.vector.tensor_add(num[:], p[:, 0], p[:, 1])
        for c in range(2, 9):
            nc.vector.tensor_mul(p3(c % 2), e[:, c], u4(k, c))
            nc.vector.tensor_add(num[:], num[:], p[:, c % 2])

    def norm_chunk(k):
        outt = out_pool.tile([128, 4, FCH], BF16, tag="outt", name=f"outt{k}")
        nc.vector.tensor_mul(
            outt[:].rearrange("p a b -> p (a b)"), nums[k][:], r4s[k][:]
        )
        nc.sync.dma_start(out_v[:, :, k, :], outt[:])

    # ---- schedule ---------------------------------------------------------
    # scalar queue: spg triggers with a 2-transfer lead; all k2 triggers go
    # out before the texp/urep ops (which block on DVE) so the k2 transfers
    # stream during the topk instead of after it.
    for c in range(2):
        spg_trigger(0, c)
    for c in range(9):
        if c + 2 < 9:
            spg_trigger(0, c + 2)
        else:
            spg_trigger(1, c + 2 - 9)
        exp_one(0, c)
    for c in range(9):
        if c + 2 < 9:
            spg_trigger(1, c + 2)
        else:
            spg_trigger(2, c + 2 - 9)
        exp_one(1, c)
    for c in range(2, 9):
        spg_trigger(2, c)

    maxes(0, 64)
    den_chunk(0, DEN_ENG[0])  # Pool, overlaps DVE topk
    maxes(64, HALF_W)
    disp4_a(0)
    texp_half(0)
    for c in range(0, 4):
        exp_one(2, c)
    maxes(HALF_W, 192)
    disp4_b(0)
    urep_half(0)
    den_chunk(1, DEN_ENG[1])
    maxes(192, W)
    disp4_a(1)
    texp_half(1)
    for c in range(4, 9):
        exp_one(2, c)
    disp4_b(1)
    urep_half(1)
    den_chunk(2, DEN_ENG[2])
    recip_chunk(0)
    recip_chunk(1)
    num_chunk(0)
    norm_chunk(0)
    num_chunk(1)
    recip_chunk(2)
    norm_chunk(1)
    num_chunk(2)
    norm_chunk(2)


def build_program():
    nc = bacc.Bacc(
        "TRN2",
        target_bir_lowering=False,
        debug=False,
        enable_asserts=False,
        num_devices=N_CORES,
    )
    cost_d = nc.dram_tensor("cost", [D, H, W], F32, kind="ExternalInput").ap()
    spg_d = nc.dram_tensor("spg", [9, HF, WF], F32, kind="ExternalInput").ap()
    out_d = nc.dram_tensor("out", [HF, WF], BF16, kind="ExternalOutput").ap()
    with tile.TileContext(nc) as tc:
        with ExitStack() as ctx:
            build_kernel(ctx, tc, out_d, cost_d, spg_d)
    nc.compile()
    return nc


def _install_ntff_hook():
    """Provide antenv.axon_hooks + register the ctypes NTFF profiler."""
    import types

    if "antenv.axon_hooks" in sys.modules:
        return True
    try:
        import antenv
        from trn_agent_boot.trn_boot import _ntff_profile_via_ctypes

        mod = types.ModuleType("antenv.axon_hooks")
        mod._hook = None

        def set_axon_ntff_profile_hook(hook):
            mod._hook = hook

        def get_axon_ntff_profile_hook():
            return mod._hook

        mod.set_axon_ntff_profile_hook = set_axon_ntff_profile_hook
        mod.get_axon_ntff_profile_hook = get_axon_ntff_profile_hook
        sys.modules["antenv.axon_hooks"] = mod
        antenv.axon_hooks = mod
        mod._hook = _ntff_profile_via_ctypes("/opt/axon/libaxon_pjrt.so")
        return True
    except Exception as e:  # profiling is best-effort
        print(f"NTFF hook install failed: {e}")
        return False


LAST_RESULTS = None


def kernel(cost: np.ndarray, spg: np.ndarray) -> np.ndarray:
    """cost [8,1,48,128,240] f32, spg [8,9,512,960] f32 -> disp1 [8,512,960] f32."""
    global LAST_RESULTS
    cost = np.ascontiguousarray(np.asarray(cost, dtype=np.float32))
    spg = np.ascontiguousarray(np.asarray(spg, dtype=np.float32))
    assert cost.shape == (B, 1, D, H, W) and spg.shape == (B, 9, HF, WF)

    nc = build_program()
    in_maps = [
        {"cost": cost[b, 0], "spg": spg[b]} for b in range(B)
    ]
    trace = bool(int(os.environ.get("KERNEL_TRACE", "0")))
    if trace:
        trace = _install_ntff_hook()
    res = run_bass_kernel_spmd(
        nc, in_maps, core_ids=list(range(N_CORES)), trace=trace
    )
    LAST_RESULTS = res
    out = np.stack(
        [np.asarray(res.results[b]["out"]) for b in range(B)], axis=0
    )
    return out.astype(np.float32, copy=False)
